# revision 18
# baseline (speedup 1.0000x reference)
"""Trainium2 Bass kernel for ANI-1x angular terms (P=2M pairs -> (P, 32)).

Data-parallel over pairs: 8 cores x 250k pairs (padded to 251904 = 128*1968).
Host supplies bf16 component planes [6, 128, T] per core (x0,y0,z0,x1,y1,z1);
device emits (32, NP_PAD) bf16, host transposes/upcasts while unsharding.

Math (per pair), structured to balance ACT/DVE/GpSimd engines:
  n_j = |v_j|^2 via custom DVE ops  SQSUM2 (x^2+y^2) + SQADD (z^2 + prev)
  dot = sum v0*v1 (DVE mul + 2 adds, fp32)
  d_j = Sqrt(n_j)                    [ACT sqrt table]
  lq  = 1/(d0*d1) via DVE reciprocal_approx_fast
  c   = 0.95*cos(angle) = 0.95*dot*lq
  sa  = Sqrt(0.5-0.475c) = sin(angle/2); sb = Sqrt(0.5+0.475c) = cos(angle/2)
  gg_s = cos((angle-z_s)/2) = cos(z_s/2)*sb + sin(z_s/2)*sa   [custom LINCOMB]
  f1_s = gg_s^(2*zeta) = Exp(2*zeta*Ln(gg_s))   [packed 4-wide Ln/Exp chunks]
  fc(d) = 1 - Sin(pi*d/7)^2;  fcj2 = 2*fc(d0)*fc(d1)          [trig table]
  f2_a = Exp(-(se*dmean - se*ShfA_a)^2); for uniform ShfA via the recurrence
         f2_{a+1} = f2_a * r * e^{-(2a+1)D^2}, r = e^{2D w}   [GpSimd stt]
  out[a*8+s] = f1_s * (f2_a * fcj2)   [bf16 muls on DVE + some GpSimd]
"""


import math
import sys

import numpy as np

try:
    import concourse.bass as bass
except ImportError:  # fresh grading dir may not have the repo on sys.path
    sys.path.insert(0, "/opt/trn_rl_repo")
    import concourse.bass as bass

import ml_dtypes
import concourse.tile as tile
from concourse import bacc
from concourse import mybir
from concourse.bass_utils import run_bass_kernel_spmd

P_TOTAL = 2_000_000
N_CORES = 8
PC = P_TOTAL // N_CORES  # 250_000 pairs per core
T = 1968                 # free-dim columns per partition (128*T = padded pairs)
NP_PAD = 128 * T         # 251_904
H = 2                    # column parts pipelined A->C
TP = T // H              # 984
NQ = 2                   # input DMA pieces per part
TQ = TP // NQ            # 492

F32 = mybir.dt.float32
BF16 = mybir.dt.bfloat16

LAST_RESULT = None  # set by kernel(); test.py reads exec_time_ns from here

_REG = {}


def _custom_ops():
    """Register kernel-local custom DVE ops with concourse's op registry
    (the documented extension point: define a DveOp, append to OPS)."""
    if _REG:
        return _REG
    import concourse.dve_ops as dmod
    from concourse.dve_spec import Spec, Src0, Src1, C0, C1, lower, _has_src1, sq
    from concourse.dve_uop import DveOpSpec

    defs = {
        # out = in0^2 + in1^2
        "SQSUM2_ANT": Spec(
            body=sq(Src0) + sq(Src1),
            reference=lambda in0, in1, s0, s1, imm2: (
                in0.astype(np.float32) ** 2 + in1.astype(np.float32) ** 2
            ),
        ),
        # out = in0^2 + in1
        "SQADD_ANT": Spec(
            body=sq(Src0) + Src1,
            reference=lambda in0, in1, s0, s1, imm2: (
                in0.astype(np.float32) ** 2 + in1.astype(np.float32)
            ),
        ),
        # out = in0*s0 + in1*s1
        "LINCOMB_ANT": Spec(
            body=Src0 * C0 + Src1 * C1,
            reference=lambda in0, in1, s0, s1, imm2: (
                in0.astype(np.float32) * s0 + in1.astype(np.float32) * s1
            ),
        ),
    }
    by_name = {o.name: o for o in dmod.OPS}
    for name, spec in defs.items():
        if name in by_name:
            _REG[name] = by_name[name]
            continue
        row = dmod._CUSTOM_DVE_ROW_BASE + len(dmod.OPS)
        assert row < 0x20
        dmod._SUB_OPCODE_FOR_NAME[name] = row
        shas = {}
        for ver in ("v3", "v4"):
            uops = lower(spec, ver=ver)
            shas[ver] = DveOpSpec(
                name=name, opcode=row, uops=uops, rd1_en=_has_src1(spec)
            ).sha(ver)
        op = dmod.DveOp(name, spec, subdim=False, uops_sha=shas)
        dmod.OPS.append(op)
        dmod.CUSTOM_DVE_SPECS[name] = spec
        _REG[name] = op
    return _REG


def _build(eta: float, zeta: float, shfa, shfz):
    A = mybir.ActivationFunctionType
    Op = mybir.AluOpType
    PI = math.pi
    se = math.sqrt(eta)
    ops = _custom_ops()
    SQSUM2, SQADD, LINCOMB = (
        ops["SQSUM2_ANT"], ops["SQADD_ANT"], ops["LINCOMB_ANT"],
    )

    das = [shfa[a + 1] - shfa[a] for a in range(3)]
    uniform_a = max(das) - min(das) < 1e-5
    Da = se * (shfa[1] - shfa[0]) if uniform_a else None

    nc = bacc.Bacc("TRN2", target_bir_lowering=False)
    vin = nc.declare_dram_parameter("vplanes", [6, 128, T], BF16, isOutput=False)
    out = nc.declare_dram_parameter("out", [32, NP_PAD], BF16, isOutput=True)
    v_h = vin.rearrange("c q t -> q c t")            # [128, 6, T]
    out_h = out.rearrange("k (q t) -> q k t", q=128)  # [128, 32, T]

    # Bias constants used by activation ops (bias must be a const AP in SBUF).
    K_SIN = 6  # gg rows computed on ACT via Sin(arctan-angle + bias)
    bias_list = [0.5]
    if uniform_a:
        bias_list += [-se * float(shfa[0])]
        bias_list += [
            -2.0 * Da * se * float(shfa[0]) - (2 * a + 1) * Da * Da
            for a in range(3)
        ]
    else:
        bias_list += [-se * float(a_) for a_ in shfa]
    bias_list += [math.pi / 2.0 - float(shfz[s]) / 2.0 for s in range(8 - K_SIN, 8)]
    bias_vals = []
    for bv in bias_list:
        if (F32, bv) not in nc.const_aps.aps and bv not in bias_vals:
            bias_vals.append(bv)
    const_np = np.tile(np.asarray(bias_vals, dtype=np.float32), (128, 1))
    const_dram = nc.inline_tensor(const_np, name="bias_consts")

    with tile.TileContext(nc) as tc:
        from contextlib import ExitStack
        from concourse.tile import add_dep_helper

        # Chain every ACT op to the previous one so the list scheduler cannot
        # interleave table phases (keeps act-table loads at 3 per part).
        last_act = [None]

        def act(*args, **kwargs):
            inst = nc.scalar.activation(*args, **kwargs)
            raw = getattr(inst, "ins", inst)
            if last_act[0] is not None:
                add_dep_helper(raw, last_act[0], reason="act-table order pin")
            last_act[0] = raw
            return inst

        with ExitStack() as ctx:
            pConst = ctx.enter_context(tc.tile_pool(name="pConst", bufs=1))
            ctile = pConst.tile([128, len(bias_vals)], F32, tag="consts")
            cdma = [False]

            def load_consts():
                nc.sync.dma_start(out=ctile[:], in_=const_dram[:])
                cdma[0] = True
            for i, bv in enumerate(bias_vals):
                nc.const_aps.aps[(F32, bv)] = ctile[:, i : i + 1]

            pV = ctx.enter_context(tc.tile_pool(name="pV", bufs=3))
            pN = ctx.enter_context(tc.tile_pool(name="pN", bufs=2))
            pPR = ctx.enter_context(tc.tile_pool(name="pPR", bufs=2))
            pDot = ctx.enter_context(tc.tile_pool(name="pDot", bufs=2))
            pD01 = ctx.enter_context(tc.tile_pool(name="pD01", bufs=1))
            pSc = ctx.enter_context(tc.tile_pool(name="pSc", bufs=1))
            pGG = ctx.enter_context(tc.tile_pool(name="pGG", bufs=1))
            pF1 = ctx.enter_context(tc.tile_pool(name="pF1", bufs=2))
            pG = ctx.enter_context(tc.tile_pool(name="pG", bufs=1))
            pOut = ctx.enter_context(tc.tile_pool(name="pOut", bufs=3))

            def emit_geom(h):
                st = {}
                st["n01"] = n01 = pN.tile([128, 2, TP], BF16, tag="n01",
                                          name=f"n01_{h}")
                st["dot"] = dot = pDot.tile([128, TP], F32, tag="dot",
                                            name=f"dot_{h}")
                # part 0 starts with a small piece so compute begins sooner
                qws = [164, 328, TQ] if h == 0 else [TQ, TQ]
                qoff = 0
                for q, qw in enumerate(qws):
                    qs = slice(qoff, qoff + qw)
                    c0 = h * TP + qoff
                    qoff += qw
                    V = pV.tile([128, 6, qw], BF16, tag="v", name=f"V_{h}_{q}")
                    nc.sync.dma_start(out=V[:], in_=v_h[:, :, c0 : c0 + qw])
                    if not cdma[0]:
                        load_consts()
                    Vf = V[:]
                    nc.vector._custom_dve(
                        SQSUM2, out=n01[:, :, qs],
                        in0=Vf[:, 0::3, :], in1=Vf[:, 1::3, :],
                    )
                    nc.vector._custom_dve(
                        SQADD, out=n01[:, :, qs],
                        in0=Vf[:, 2::3, :], in1=n01[:, :, qs],
                    )
                    PR = pPR.tile([128, 3, qw], F32, tag="pr", name=f"PR_{h}_{q}")
                    nc.vector.tensor_mul(PR[:], Vf[:, 0:3, :], Vf[:, 3:6, :])
                    nc.vector.tensor_add(dot[:, qs], PR[:, 0, :], PR[:, 1, :])
                    nc.vector.tensor_add(dot[:, qs], dot[:, qs], PR[:, 2, :])
                return st

            def emit_sqrt_head(h, st):
                st["d01"] = d01 = pD01.tile([128, 2, TP], F32, tag="d01",
                                            name=f"d01_{h}")
                act(d01[:], st["n01"][:], A.Sqrt)

            def emit_mid(h, st):
                d01 = st["d01"]
                dot = st["dot"]
                dd = pSc.tile([128, TP], F32, tag="dd", name=f"dd_{h}")
                nc.vector.tensor_mul(dd[:], d01[:, 0, :], d01[:, 1, :])
                lq = pSc.tile([128, TP], F32, tag="lq", name=f"lq_{h}")
                nc.vector.reciprocal_approx_fast(out=lq[:], in_=dd[:])
                # c = 0.95*dot*lq, in place over dot
                nc.vector.scalar_tensor_tensor(
                    dot[:], dot[:], 0.95, lq[:], op0=Op.mult, op1=Op.mult
                )
                st["sa"] = sa = pSc.tile([128, TP], F32, tag="sa", name=f"sa_{h}")
                st["sb"] = sb = pSc.tile([128, TP], F32, tag="sb", name=f"sb_{h}")
                act(sa[:], dot[:], A.Sqrt, scale=-0.5, bias=0.5)
                act(sb[:], dot[:], A.Sqrt, scale=0.5, bias=0.5)
                st["dm"] = dm = pSc.tile([128, TP], F32, tag="dm", name=f"dm_{h}")
                nc.gpsimd.tensor_add(dm[:], d01[:, 0, :], d01[:, 1, :])
                # t = tan(angle/2) = sa/sb, for the ACT-side gg rows
                rsb = pSc.tile([128, TP], F32, tag="rsb", name=f"rsb_{h}")
                nc.vector.reciprocal_approx_fast(out=rsb[:], in_=sb[:])
                st["tn"] = tn = pSc.tile([128, TP], F32, tag="tn", name=f"tn_{h}")
                nc.vector.tensor_mul(tn[:], sa[:], rsb[:])

            def emit_lincomb(h, st, s_lo, s_hi):
                if "gg" not in st:
                    st["gg"] = pGG.tile([128, 8, TP], F32, tag="gg",
                                        name=f"gg_{h}")
                gg = st["gg"]
                for s in range(s_lo, min(s_hi, 8 - K_SIN)):
                    c1 = math.cos(float(shfz[s]) / 2.0)
                    s1 = math.sin(float(shfz[s]) / 2.0)
                    nc.vector._custom_dve(
                        LINCOMB, out=gg[:, s, :], in0=st["sb"][:],
                        in1=st["sa"][:], s0=c1, s1=s1,
                    )

            def emit_trig(h, st):
                # gg_s = cos(angle/2 - z_s/2) = Sin(arctan(t) + pi/2 - z_s/2)
                gg = st["gg"]
                om = pSc.tile([128, TP], F32, tag="om", name=f"om_{h}")
                act(om[:], st["tn"][:], A.Arctan)
                for s in range(8 - K_SIN, 8):
                    act(gg[:, s, :], om[:], A.Sin,
                        bias=math.pi / 2.0 - float(shfz[s]) / 2.0)
                st["sfc"] = sfc = pG.tile([128, 2, TP], BF16, tag="sfc",
                                          name=f"sfc_{h}")
                act(sfc[:], st["d01"][:], A.Sin, scale=PI / 7.0)
                act(sfc[:], sfc[:], A.Square)

            def emit_f2(h, st):
                dm = st["dm"]
                st["f2"] = f2 = pG.tile([128, 4, TP], BF16, tag="f2",
                                        name=f"f2_{h}")
                q0 = pG.tile([128, TP], F32, tag="q0", name=f"q0_{h}")
                act(q0[:], dm[:], A.Square, scale=se / 2.0,
                    bias=-se * float(shfa[0]))
                act(f2[:, 0, :], q0[:], A.Exp, scale=-1.0)
                if uniform_a:
                    st["r"] = r = pG.tile([128, 3, TP], BF16, tag="r",
                                          name=f"r_{h}")
                    for a in range(3):
                        bias_a = (-2.0 * Da * se * float(shfa[0])
                                  - (2 * a + 1) * Da * Da)
                        act(r[:, a, :], dm[:], A.Exp, scale=Da * se,
                            bias=bias_a)
                else:
                    for a in range(1, 4):
                        act(q0[:], dm[:], A.Square, scale=se / 2.0,
                            bias=-se * float(shfa[a]))
                        act(f2[:, a, :], q0[:], A.Exp, scale=-1.0)

            def emit_fcj_g2(h, st):
                sfc, f2 = st["sfc"], st["f2"]
                ff = pG.tile([128, TP], BF16, tag="ff", name=f"ff_{h}")
                nc.vector.tensor_scalar(
                    ff[:], sfc[:, 0, :], 2.0, 2.0, op0=Op.mult, op1=Op.subtract
                )
                fcj = pG.tile([128, TP], BF16, tag="fcj", name=f"fcj_{h}")
                nc.vector.scalar_tensor_tensor(
                    fcj[:], sfc[:, 1, :], 1.0, ff[:], op0=Op.subtract, op1=Op.mult
                )
                st["g2"] = g2 = pG.tile([128, 4, TP], BF16, tag="g2",
                                        name=f"g2_{h}")
                if uniform_a:
                    # fold fcj2 into the f2 recurrence: g2_0 = f2_0*fcj2,
                    # g2_{a+1} = r_a * g2_a  (the fcj2 factor rides along)
                    r = st["r"]
                    nc.vector.tensor_mul(g2[:, 0, :], f2[:, 0, :], fcj[:])
                    for a in range(3):
                        nc.vector.tensor_mul(
                            g2[:, a + 1, :], r[:, a, :], g2[:, a, :]
                        )
                else:
                    for a in range(4):
                        nc.vector.tensor_mul(g2[:, a, :], f2[:, a, :], fcj[:])

            def emit_lnexp(h, st, k):
                gg = st["gg"]
                if "f1" not in st:
                    st["f1"] = pF1.tile([128, 8, TP], BF16, tag="f1",
                                        name=f"f1_{h}")
                cs = slice(4 * k, 4 * k + 4)
                act(gg[:, cs, :], gg[:, cs, :], A.Ln)
                act(st["f1"][:, cs, :], gg[:, cs, :], A.Exp, scale=2.0 * zeta)

            def emit_finals(h, st, k):
                f1, g2 = st["f1"], st["g2"]
                col0 = h * TP
                cols = slice(col0, col0 + TP)
                for s in range(4 * k, 4 * k + 4):
                    ot = pOut.tile([128, 4, TP], BF16, tag="out", bufs=3,
                                   name=f"ot_{h}_{s}")
                    f1b = f1[:, s, :].unsqueeze(1).broadcast_to([128, 4, TP])
                    nc.vector.tensor_mul(ot[:], f1b, g2[:])
                    last = h == H - 1 and s == 7
                    na = 1 if last else 4
                    for a0 in range(0, 4, na):
                        nc.sync.dma_start(
                            out=out_h[:, 8 * a0 + s : 8 * (a0 + na - 1) + s + 1 : 8,
                                      cols],
                            in_=ot[:, a0 : a0 + na, :],
                        )

            st0 = emit_geom(0)
            emit_sqrt_head(0, st0)
            emit_mid(0, st0)
            emit_lincomb(0, st0, 0, 8)
            st1 = emit_geom(1)
            emit_trig(0, st0)
            emit_f2(0, st0)
            emit_fcj_g2(0, st0)
            emit_lnexp(0, st0, 0)
            emit_finals(0, st0, 0)
            emit_lnexp(0, st0, 1)
            emit_sqrt_head(1, st1)
            emit_mid(1, st1)
            emit_finals(0, st0, 1)
            emit_lincomb(1, st1, 0, 8)
            emit_trig(1, st1)
            emit_f2(1, st1)
            emit_fcj_g2(1, st1)
            emit_lnexp(1, st1, 0)
            emit_finals(1, st1, 0)
            emit_lnexp(1, st1, 1)
            emit_finals(1, st1, 1)

    nc.finalize()
    _fix_act_table_loads(nc)
    return nc


def _fix_act_table_loads(nc):
    """Replace Bacc's per-function act-table loads with a minimal greedy
    assignment: at each point where the current set no longer covers the
    next activation, pick the set covering the longest upcoming run."""
    from concourse.hw_specs import get_activation_tables

    tables = list(get_activation_tables(nc.m.arch).items())
    name_to_id = {n: i for i, (n, _) in enumerate(tables)}
    sets = dict(tables)
    prefer = ["sqrt_and_others", "trig_and_small", "natural_log_exp_and_others"]
    for b in nc.m.functions[0].blocks:
        insts = b.instructions
        loads = [i for i in insts if type(i).__name__ == "InstLoadActFuncSet"]
        if not loads:
            continue
        for ld in loads:
            insts.remove(ld)
        acts = [i for i in insts if isinstance(i, mybir.InstActivation)]
        plan = []
        cur = None
        for idx, ins_ in enumerate(acts):
            fn = ins_.func
            if cur is not None and fn in sets[cur]:
                continue
            best, bestlen = None, -1
            for n in prefer:
                if fn not in sets[n]:
                    continue
                L = 0
                for j in range(idx, len(acts)):
                    if acts[j].func in sets[n]:
                        L += 1
                    else:
                        break
                if L > bestlen:
                    best, bestlen = n, L
            if best is None:
                for n, s in tables:
                    if fn in s:
                        best = n
                        break
            assert best is not None, f"no act table covers {fn}"
            plan.append((ins_, best))
            cur = best
        assert len(plan) <= len(loads), (len(plan), len(loads))
        spare = list(loads)
        for anchor, set_name in plan:
            ld = spare.pop()
            ld.act_func_set_id = name_to_id[set_name]
            insts.insert(insts.index(anchor), ld)


_BUILD_CACHE = {}


def kernel(vectors12, EtaA, Zeta, ShfA, ShfZ, _trace=False):
    global LAST_RESULT
    eta = float(np.asarray(EtaA).reshape(-1)[0])
    zeta = float(np.asarray(Zeta).reshape(-1)[0])
    shfa = [float(x) for x in np.asarray(ShfA).reshape(-1)]
    shfz = [float(x) for x in np.asarray(ShfZ).reshape(-1)]
    assert len(shfa) == 4 and len(shfz) == 8

    key = (eta, zeta, tuple(shfa), tuple(shfz))
    nc = _BUILD_CACHE.get(key)
    if nc is None:
        nc = _build(eta, zeta, shfa, shfz)
        _BUILD_CACHE[key] = nc

    v = np.asarray(vectors12, dtype=np.float32)
    assert v.shape == (2, P_TOTAL, 3)
    in_maps = []
    for i in range(N_CORES):
        shard = np.ones((2, NP_PAD, 3), dtype=np.float32)
        shard[:, :PC, :] = v[:, i * PC : (i + 1) * PC, :]
        planes = np.ascontiguousarray(
            shard.reshape(2, 128, T, 3).transpose(0, 3, 1, 2)
        ).reshape(6, 128, T)
        in_maps.append({"vplanes": planes.astype(ml_dtypes.bfloat16)})

    res = run_bass_kernel_spmd(nc, in_maps, core_ids=list(range(N_CORES)),
                               trace=_trace)
    LAST_RESULT = res

    full = np.empty((P_TOTAL, 32), dtype=np.float32)
    for i in range(N_CORES):
        o = res.results[i]["out"]  # (32, NP_PAD) bf16
        full[i * PC : (i + 1) * PC, :] = o[:, :PC].T.astype(np.float32)
    return full
